# revision 27
# baseline (speedup 1.0000x reference)
"""Brute-force KNN (B=2, Ns=16384, Nq=8192, d=3, k<=16) on 8 trn2 NeuronCores.

Strategy (data-parallel over queries):
  - 16384 total queries sharded 2048/core (cores 0-3: batch 0, cores 4-7: batch 1).
  - PE computes score[q,s] = q . s - ||s||^2/2 (rank-equivalent to -d2/2) via
    a K=11 fp16 matmul built from hi/lo residual splits of the coordinates
    and the ||s||^2 term, so the fp32 PSUM scores are exact to ~1e-6 for the
    fp32 inputs (matmul time on this hw is K-independent). 2048 support
    columns per chunk (psum [128,2048]f32 x2 = all 8 banks), 256-col matmul
    instructions (measured 2x the per-column rate of 512-col ones).
  - Top-k on trn2 is bottlenecked by reading PSUM: Max/MaxIndex run at
    1 elem/lane/cycle with no fast modes, and TensorTensor cannot read two
    PSUM operands. So the Scalar engine (Activation Copy, ~0.83ns/elem)
    drains most psum chunks to SBUF fp16 and the DVE max-folds 32:1 down to
    128 fp16 slot maxima per 4096-col window (2-byte 2x mode); for ~12 of 64
    windows the DVE instead max-folds the raw psum chunk against the
    neighbor chunk's copy (mixed fold, 1 elem/cycle), balancing DVE vs
    Scalar. Fold ops are merged across chunk pairs and pair "supers" via
    strided APs to amortize per-instruction overhead. Max/MaxIndex then pick
    the top-8 slots of each 64-slot half-window (64 candidates/query).
  - Each slot names a group of 32 support columns (col = win*4096 + slot +
    128*j). The host exact-reranks all 64*32=2048 columns in fp32 - fold
    losers are recovered because a fold winner always beats its group, so
    any true neighbor's group winner is itself a top candidate.
  - Conservative host fallbacks (full-row exact rerank) for: a half-window
    holding >=8 of the found top-k, slot-boundary within fold-fp16 noise
    (delta=0.004) of the found k-th distance, duplicated slots near the
    boundary.
"""

import numpy as np

import concourse.bass as bass
from concourse import mybir
from concourse.bass_utils import run_bass_kernel_spmd

B = 2
NS = 16384
NQ = 8192
N_CORES = 8
QPC = (B * NQ) // N_CORES  # queries per core = 2048
N_TILES = QPC // 128  # 16
KDIM = 11  # hi/lo residual rows
CHUNK = 2048  # psum chunk ([128,2048] fp32 = 4 banks, x2 buffers = all PSUM)
PAIR = 2 * CHUNK  # candidate window: two psum chunks fold into one 4096-window
PAIRS_PER_TILE = NS // PAIR  # 4
N_CHUNKS = N_TILES * (NS // CHUNK)  # 128
N_PAIRS = N_CHUNKS // 2  # 64
N_SUPERS = N_PAIRS // 2  # 32 (two pairs share merged downstream folds)
FOLD = 32  # 4096-window -> 128 slots
NSLOT = PAIR // FOLD  # 128
NCAND = PAIRS_PER_TILE * 16  # 64 slots per query (top-8 of each 64-slot half)

# Mixed pairs: DVE reads the even psum chunk directly (folded against the odd
# chunk's SBUF copy) instead of Scalar copying it. With only 2 psum buffers
# the PE->Act->DVE->PE recycle latency of a mixed pair stalls PE more than it
# saves on Scalar, so this stays 0.
M_COUNT = 0


def _is_m(p: int) -> bool:
    return (p * M_COUNT) // N_PAIRS != ((p + 1) * M_COUNT) // N_PAIRS


# chunk k is Scalar-copied unless it's the even chunk of a mixed pair
def _copied(k: int) -> bool:
    return (k % 2 == 1) or not _is_m(k // 2)


_CUM_COPIES = []
_c = 0
for _k in range(N_CHUNKS):
    _c += 1 if _copied(_k) else 0
    _CUM_COPIES.append(_c)

LAST_RESULTS = None  # stashed BassKernelResults for test harness introspection


def _build_program():
    nc = bass.Bass()
    lhsT = nc.declare_dram_parameter(
        "lhsT", [KDIM, QPC], mybir.dt.float16, isOutput=False
    )
    rhs = nc.declare_dram_parameter("rhs", [KDIM, NS], mybir.dt.float16, isOutput=False)
    out_idx = nc.declare_dram_parameter(
        "out_idx", [QPC, NCAND], mybir.dt.uint16, isOutput=True
    )

    from contextlib import ExitStack

    with ExitStack() as stack:
        _n = [0]

        def sb(shape, dt):
            _n[0] += 1
            return stack.enter_context(nc.sbuf_tensor(f"sb{_n[0]}", shape, dt))

        lhs_sb = sb([KDIM, QPC], mybir.dt.float16)
        rhs_sb = sb([KDIM, NS], mybir.dt.float16)
        psum = stack.enter_context(
            nc.psum_tensor("ps", [128, PAIR], mybir.dt.float32)
        )
        cbuf = [sb([128, PAIR], mybir.dt.float16) for _ in range(4)]  # per-pair ring
        f1 = [sb([128, 4096], mybir.dt.float16) for _ in range(2)]  # per-super ring
        f2 = [sb([128, 2048], mybir.dt.float16) for _ in range(2)]
        f3 = [sb([128, 1024], mybir.dt.float16) for _ in range(2)]
        f4 = [sb([128, 512], mybir.dt.float16) for _ in range(2)]
        f5 = [sb([128, 256], mybir.dt.float16) for _ in range(2)]
        v8a = [sb([128, 8], mybir.dt.float16) for _ in range(2)]
        v8b = [sb([128, 8], mybir.dt.float16) for _ in range(2)]
        i8 = [sb([128, NCAND], mybir.dt.uint16) for _ in range(2)]
        junk = sb([128, 8], mybir.dt.float16)
        sem = lambda name: stack.enter_context(nc.semaphore(name))
        dma_in = sem("dma_in")
        rp = [sem(f"rp{i}") for i in range(4)]
        pe_sem = sem("pe_sem")
        act_sem = sem("act_sem")
        dve_ing = sem("dve_ing")
        out_sem = sem("out_sem")
        dma_out = sem("dma_out")
        block = stack.enter_context(nc.Block())

        def pair3(ap, h):
            """[128, 2*n] flat -> [128, 2, h] strided view (n = ap cols / 2)."""
            return ap.rearrange("p (c h) -> p c h", c=2)[:, :, 0:h]

        @block.sync
        def _(sync):
            sync.dma_start(lhs_sb[:], lhsT[:]).then_inc(dma_in, 16)
            for piece in range(4):
                w = NS // 4
                sync.dma_start(
                    rhs_sb[:, piece * w : (piece + 1) * w],
                    rhs[:, piece * w : (piece + 1) * w],
                ).then_inc(rp[piece], 16)
            for t in range(N_TILES):
                sync.wait_ge(out_sem, t + 1)
                sync.dma_start(
                    out_idx[t * 128 : (t + 1) * 128, :], i8[t % 2][:]
                ).then_inc(dma_out, 16)

        @block.tensor
        def _(tensor):
            tensor.wait_ge(dma_in, 16)
            for k in range(N_CHUNKS):
                t = k // (NS // CHUNK)
                c = k % (NS // CHUNK)
                if k < 2 * (NS // CHUNK):
                    tensor.wait_ge(rp[c // 2], 16)
                if k >= 2:
                    # psum half (k%2) free when pair (k-2)//2's copy is done
                    tensor.wait_ge(act_sem, (k - 2) // 2 + 1)
                lt = lhs_sb[:, t * 128 : (t + 1) * 128]
                pt = psum[:, (k % 2) * CHUNK : (k % 2 + 1) * CHUNK]
                for j in range(CHUNK // 256):
                    ins = nc.tensor.matmul(
                        pt[:, j * 256 : (j + 1) * 256],
                        lt,
                        rhs_sb[:, c * CHUNK + j * 256 : c * CHUNK + (j + 1) * 256],
                        start=True,
                        stop=True,
                    )
                    if j == CHUNK // 256 - 1:
                        ins.then_inc(pe_sem, 1)

        @block.scalar
        def _(scalar):
            for p in range(N_PAIRS):
                scalar.wait_ge(pe_sem, 2 * p + 2)
                if p >= 4:
                    scalar.wait_ge(dve_ing, p - 3)  # cbuf[p%4] consumer done
                nc.scalar.activation(
                    cbuf[p % 4][:],
                    psum[:],
                    mybir.ActivationFunctionType.Copy,
                ).then_inc(act_sem, 1)

        @block.vector
        def _(vector):
            def fold1(p):
                """ingest pair p (2 chunks) -> f1[super%2] pair-slot (2048)."""
                s = p // 2
                vector.wait_ge(act_sem, p + 1)
                cb = cbuf[p % 4]
                out = pair3(
                    f1[s % 2][:, (p % 2) * 2048 : (p % 2 + 1) * 2048], 1024
                )
                # fold each chunk's halves: [c, 0:1024] vs [c, 1024:2048]
                in0 = pair3(cb[:, 0:PAIR], 1024)
                in1 = cb[:, 0:PAIR].rearrange("p (c h) -> p c h", c=2)[
                    :, :, 1024:2048
                ]
                nc.vector.tensor_max(out, in0, in1).then_inc(dve_ing, 1)

            def downstream(s):
                """fold super s's 2x2048 fp16 -> 2x128 slot maxima."""
                a = f1[s % 2][:, 0:4096].rearrange("p (j h) -> p j h", j=2)
                o = f2[s % 2][:, 0:2048].rearrange("p (j h) -> p j h", j=2)
                nc.vector.tensor_max(o, a[:, :, 0:1024], a[:, :, 1024:2048])
                a = f2[s % 2][:, 0:2048].rearrange("p (j h) -> p j h", j=2)
                o = f3[s % 2][:, 0:1024].rearrange("p (j h) -> p j h", j=2)
                nc.vector.tensor_max(o, a[:, :, 0:512], a[:, :, 512:1024])
                a = f3[s % 2][:, 0:1024].rearrange("p (j h) -> p j h", j=2)
                o = f4[s % 2][:, 0:512].rearrange("p (j h) -> p j h", j=2)
                nc.vector.tensor_max(o, a[:, :, 0:256], a[:, :, 256:512])
                a = f4[s % 2][:, 0:512].rearrange("p (j h) -> p j h", j=2)
                o = f5[s % 2][:, 0:256].rearrange("p (j h) -> p j h", j=2)
                nc.vector.tensor_max(o, a[:, :, 0:128], a[:, :, 128:256])

            def maxes(p):
                s = p // 2
                base = (p % 2) * 128
                nc.vector.max(v8a[p % 2][:], f5[s % 2][:, base : base + 64])
                nc.vector.max(v8b[p % 2][:], f5[s % 2][:, base + 64 : base + 128])

            def mindex(p):
                s = p // 2
                t = p // PAIRS_PER_TILE
                c = p % PAIRS_PER_TILE
                base = (p % 2) * 128
                ib = i8[t % 2]
                nc.vector.max_index(
                    ib[:, c * 16 : c * 16 + 8],
                    v8a[p % 2][:],
                    f5[s % 2][:, base : base + 64],
                )
                ins = nc.vector.max_index(
                    ib[:, c * 16 + 8 : c * 16 + 16],
                    v8b[p % 2][:],
                    f5[s % 2][:, base + 64 : base + 128],
                )
                if c == PAIRS_PER_TILE - 1:
                    ins.then_inc(out_sem, 1)

            for s in range(N_SUPERS):
                fold1(2 * s)
                fold1(2 * s + 1)
                downstream(s)
                if s >= 1:
                    for pp in (2 * s - 2, 2 * s - 1):
                        tp = pp // PAIRS_PER_TILE
                        if pp % PAIRS_PER_TILE == 0 and tp >= 2:
                            vector.wait_ge(dma_out, 16 * (tp - 1))
                        mindex(pp)  # gap after maxes via this super's folds
                maxes(2 * s)
                maxes(2 * s + 1)
            nc.vector.tensor_copy(junk[:], f1[(N_SUPERS - 1) % 2][:, 0:8])  # gap
            mindex(N_PAIRS - 2)
            mindex(N_PAIRS - 1)

    return nc


_NC_CACHE = None


def _get_nc():
    global _NC_CACHE
    if _NC_CACHE is None:
        _NC_CACHE = _build_program()
    return _NC_CACHE


def _exact_d2_rows(q, s_all, cand):
    """Reference-matching fp32 d2 for candidate columns.

    q: (n,3) f32 queries; s_all: (NS,3) f32; cand: (n,m) int
    Returns (n,m) f32 d2 computed as (q_sq + s_sq) - 2*cross, all float32
    like the jax reference.
    """
    q_sq = (q[:, 0] * q[:, 0] + q[:, 1] * q[:, 1]) + q[:, 2] * q[:, 2]
    sc = s_all[cand]  # (n, m, 3)
    s_sq = (sc[..., 0] * sc[..., 0] + sc[..., 1] * sc[..., 1]) + sc[..., 2] * sc[..., 2]
    cross = (q[:, None, 0] * sc[..., 0] + q[:, None, 1] * sc[..., 1]) + (
        q[:, None, 2] * sc[..., 2]
    )
    return (q_sq[:, None] + s_sq) - np.float32(2.0) * cross


def _hilo(x):
    hi = x.astype(np.float16)
    lo = (x - hi.astype(np.float32)).astype(np.float16)
    return hi, lo


def kernel(xyz, xyz_query, n_neighbors):
    global LAST_RESULTS
    xyz = np.asarray(xyz, dtype=np.float32)
    xyz_query = np.asarray(xyz_query, dtype=np.float32)
    k = int(n_neighbors)
    assert k <= 16, f"k={k} too large for candidate margin"

    # --- per-core device inputs (hi/lo residual split, K=11) ---
    # score = (qh+ql) . (sh+sl) - |s_eff|^2/2 with ql*sl dropped (~1e-7):
    # rows: qh*sh (3), ql*sh (3), qh*sl (3), 1*s2h, 1*s2l
    in_maps = []
    for core in range(N_CORES):
        b = core // (N_CORES // B)
        q0 = (core % (N_CORES // B)) * QPC
        q = xyz_query[b, q0 : q0 + QPC]  # (2048, 3)
        s = xyz[b]  # (16384, 3)
        qh, ql = _hilo(q)
        sh, sl = _hilo(s)
        s_eff = sh.astype(np.float64) + sl.astype(np.float64)
        s2 = -0.5 * (s_eff * s_eff).sum(-1)
        s2h = s2.astype(np.float16)
        s2l = (s2 - s2h.astype(np.float64)).astype(np.float16)
        lhsT = np.empty((KDIM, QPC), np.float16)
        rhsm = np.empty((KDIM, NS), np.float16)
        for d in range(3):
            lhsT[d] = qh[:, d]
            rhsm[d] = sh[:, d]
            lhsT[3 + d] = ql[:, d]
            rhsm[3 + d] = sh[:, d]
            lhsT[6 + d] = qh[:, d]
            rhsm[6 + d] = sl[:, d]
        lhsT[9] = 1.0
        rhsm[9] = s2h
        lhsT[10] = 1.0
        rhsm[10] = s2l
        in_maps.append({"lhsT": lhsT, "rhs": rhsm})

    nc = _get_nc()
    res = run_bass_kernel_spmd(nc, in_maps, list(range(N_CORES)))
    LAST_RESULTS = res

    neighbors = np.empty((B, NQ, k), np.int32)
    distances = np.empty((B, NQ, k), np.float32)
    rows_fallback = 0
    stats = [0, 0, 0, 0]  # flag_a, flag_b, flag_c, any-dup counts

    n_win = NS // PAIR  # 4 candidate windows per row
    n_half = 2 * n_win  # 8 half-windows
    j = np.arange(NCAND)
    colbase = (j // 16) * PAIR + ((j % 16) // 8) * 64  # (64,)
    offs = NSLOT * np.arange(FOLD)  # (32,) offsets within a slot's group
    DELTA = np.float32(0.004)

    for core in range(N_CORES):
        b = core // (N_CORES // B)
        q0 = (core % (N_CORES // B)) * QPC
        q = xyz_query[b, q0 : q0 + QPC]
        s = xyz[b]
        r = res.results[core]
        slots = r["out_idx"].astype(np.int64)  # (2048, 64) slot in [0,64)

        # expand each slot to its 32-column fold group
        cand = (
            colbase[None, :, None] + slots[:, :, None] + offs[None, None, :]
        )  # (2048, 64, 32)
        cand2 = cand.reshape(QPC, NCAND * FOLD)
        d2 = _exact_d2_rows(q, s, cand2)  # (2048, 2048) f32

        # top-64 by d2 first (cheap), then stable (d2, idx) order among them
        part = np.argpartition(d2, 63, axis=1)[:, :64]
        d2p = np.take_along_axis(d2, part, 1)
        cp = np.take_along_axis(cand2, part, 1)
        order = np.lexsort((cp, d2p))
        cand_s = np.take_along_axis(cp, order, 1)
        d2_s = np.take_along_axis(d2p, order, 1)
        topk_idx = cand_s[:, :k]
        topk_d2 = d2_s[:, :k]

        # --- conservative fallback detection ---
        thresh = topk_d2[:, k - 1] + DELTA  # (2048,)
        # (a) a half-window contributed >=8 of the found top-k
        half_of = (topk_idx // PAIR) * 2 + (topk_idx % NSLOT) // 64
        counts = (half_of[:, :, None] == np.arange(n_half)[None, None]).sum(1)
        flag_a = counts.max(1) >= 8
        # (b) slot boundary within noise of the found k-th d2
        gmin = d2.reshape(QPC, NCAND, FOLD).min(2)  # (2048, 64) slot-group best
        boundary = gmin.reshape(QPC, n_half, 8).max(2)  # (2048, 8)
        flag_b = (boundary.min(1) - topk_d2[:, k - 1]) < DELTA
        # (c) duplicate slots within a half-window (max_index tie artifact):
        #     only matters if the dup slot's group-best is near the k-th d2
        sh_ = slots.reshape(QPC, n_half, 8)
        dup = (sh_[:, :, :, None] == sh_[:, :, None, :]).sum(3) > 1  # (q,h,8)
        gmin_h = gmin.reshape(QPC, n_half, 8)
        dup_gmin = np.where(dup, gmin_h, np.float32(np.inf)).min((1, 2))
        flag_c = dup_gmin < thresh
        flag = flag_a | flag_b | flag_c
        stats[0] += int(flag_a.sum())
        stats[1] += int(flag_b.sum())
        stats[2] += int(flag_c.sum())
        stats[3] += int(dup.any((1, 2)).sum())

        nb = topk_idx.astype(np.int32)
        dd = topk_d2

        if flag.any():
            rows = np.nonzero(flag)[0]
            rows_fallback += len(rows)
            full = _exact_d2_rows(
                q[rows], s, np.broadcast_to(np.arange(NS), (len(rows), NS))
            )
            forder = np.lexsort((np.broadcast_to(np.arange(NS), full.shape), full))
            nb[rows] = forder[:, :k].astype(np.int32)
            dd = dd.copy()
            dd[rows] = np.take_along_axis(full, forder[:, :k], 1)

        neighbors[b, q0 : q0 + QPC] = nb
        distances[b, q0 : q0 + QPC] = np.sqrt(np.maximum(dd, np.float32(0.0)))

    kernel.rows_fallback = rows_fallback
    kernel.flag_stats = tuple(stats)
    return neighbors, distances


# revision 28
# speedup vs baseline: 1.7220x; 1.7220x over previous
"""Brute-force KNN (B=2, Ns=16384, Nq=8192, d=3, k<=16) on 8 trn2 NeuronCores.

Strategy (data-parallel over queries):
  - 16384 total queries sharded 2048/core (cores 0-3: batch 0, cores 4-7: batch 1).
  - PE computes score[q,s] = q . s - ||s||^2/2 (rank-equivalent to -d2/2) via
    a K=11 fp16 matmul built from hi/lo residual splits of the coordinates
    and the ||s||^2 term, so the fp32 PSUM scores are exact to ~1e-6 for the
    fp32 inputs (matmul time on this hw is K-independent). 2048 support
    columns per chunk (psum [128,2048]f32 x2 = all 8 banks), 256-col matmul
    instructions (measured 2x the per-column rate of 512-col ones).
  - Top-k on trn2 is bottlenecked by reading PSUM: Max/MaxIndex run at
    1 elem/lane/cycle with no fast modes, and TensorTensor cannot read two
    PSUM operands. So the Scalar engine (Activation Copy, ~0.83ns/elem)
    drains most psum chunks to SBUF fp16 and the DVE max-folds 32:1 down to
    128 fp16 slot maxima per 4096-col window (2-byte 2x mode); for ~12 of 64
    windows the DVE instead max-folds the raw psum chunk against the
    neighbor chunk's copy (mixed fold, 1 elem/cycle), balancing DVE vs
    Scalar. Fold ops are merged across chunk pairs and pair "supers" via
    strided APs to amortize per-instruction overhead. Max/MaxIndex then pick
    the top-8 slots of each 64-slot half-window (64 candidates/query).
  - Each slot names a group of 32 support columns (col = win*4096 + slot +
    128*j). The host exact-reranks all 64*32=2048 columns in fp32 - fold
    losers are recovered because a fold winner always beats its group, so
    any true neighbor's group winner is itself a top candidate.
  - Conservative host fallbacks (full-row exact rerank) for: a half-window
    holding >=8 of the found top-k, slot-boundary within fold-fp16 noise
    (delta=0.004) of the found k-th distance, duplicated slots near the
    boundary.
"""

import numpy as np

import concourse.bass as bass
from concourse import mybir
from concourse.bass_utils import run_bass_kernel_spmd

B = 2
NS = 16384
NQ = 8192
N_CORES = 8
QPC = (B * NQ) // N_CORES  # queries per core = 2048
N_TILES = QPC // 128  # 16
KDIM = 11  # hi/lo residual rows
CHUNK = 2048  # psum chunk ([128,2048] fp32 = 4 banks, x2 buffers = all PSUM)
PAIR = 2 * CHUNK  # candidate window: two psum chunks fold into one 4096-window
PAIRS_PER_TILE = NS // PAIR  # 4
N_CHUNKS = N_TILES * (NS // CHUNK)  # 128
N_PAIRS = N_CHUNKS // 2  # 64
N_SUPERS = N_PAIRS // 2  # 32 (two pairs share merged downstream folds)
FOLD = 32  # 4096-window -> 128 slots
NSLOT = PAIR // FOLD  # 128
NCAND = PAIRS_PER_TILE * 16  # 64 slots per query (top-8 of each 64-slot half)

# A-pairs: for the even chunk, Scalar copies only its second half and the DVE
# max-folds the psum first half against that copy (mixed fold, scheduled
# early so the psum recycle latency stays short). Balances Scalar vs DVE.
A_COUNT = 8


def _is_a(p: int) -> bool:
    return (p * A_COUNT) // N_PAIRS != ((p + 1) * A_COUNT) // N_PAIRS


_CUM_A = []
_c = 0
for _k in range(N_CHUNKS):
    _c += 1 if (_k % 2 == 0 and _is_a(_k // 2)) else 0
    _CUM_A.append(_c)

LAST_RESULTS = None  # stashed BassKernelResults for test harness introspection


def _build_program():
    nc = bass.Bass()
    lhsT = nc.declare_dram_parameter(
        "lhsT", [KDIM, QPC], mybir.dt.float16, isOutput=False
    )
    rhs = nc.declare_dram_parameter("rhs", [KDIM, NS], mybir.dt.float16, isOutput=False)
    out_idx = nc.declare_dram_parameter(
        "out_idx", [QPC, NCAND], mybir.dt.uint16, isOutput=True
    )

    from contextlib import ExitStack

    with ExitStack() as stack:
        _n = [0]

        def sb(shape, dt):
            _n[0] += 1
            return stack.enter_context(nc.sbuf_tensor(f"sb{_n[0]}", shape, dt))

        lhs_sb = sb([KDIM, QPC], mybir.dt.float16)
        rhs_sb = sb([KDIM, NS], mybir.dt.float16)
        psum = [
            stack.enter_context(
                nc.psum_tensor(f"ps{i}", [128, CHUNK], mybir.dt.float32)
            )
            for i in range(2)
        ]
        cbuf = [sb([128, PAIR], mybir.dt.float16) for _ in range(4)]  # per-pair ring
        f1 = [sb([128, 4096], mybir.dt.float16) for _ in range(2)]  # per-super ring
        f2 = [sb([128, 2048], mybir.dt.float16) for _ in range(2)]
        f3 = [sb([128, 1024], mybir.dt.float16) for _ in range(2)]
        f4 = [sb([128, 512], mybir.dt.float16) for _ in range(2)]
        f5 = [sb([128, 256], mybir.dt.float16) for _ in range(2)]
        v8a = [sb([128, 8], mybir.dt.float16) for _ in range(2)]
        v8b = [sb([128, 8], mybir.dt.float16) for _ in range(2)]
        i8 = [sb([128, NCAND], mybir.dt.uint16) for _ in range(2)]
        junk = sb([128, 8], mybir.dt.float16)
        sem = lambda name: stack.enter_context(nc.semaphore(name))
        dma_in = sem("dma_in")
        rp = [sem(f"rp{i}") for i in range(4)]
        pe_sem = sem("pe_sem")
        act_sem = sem("act_sem")
        dve_ing = sem("dve_ing")
        dve_a = sem("dve_a")
        out_sem = sem("out_sem")
        dma_out = sem("dma_out")
        block = stack.enter_context(nc.Block())

        def pair3(ap, h):
            """[128, 2*n] flat -> [128, 2, h] strided view (n = ap cols / 2)."""
            return ap.rearrange("p (c h) -> p c h", c=2)[:, :, 0:h]

        @block.sync
        def _(sync):
            sync.dma_start(lhs_sb[:], lhsT[:]).then_inc(dma_in, 16)
            for piece in range(4):
                w = NS // 4
                sync.dma_start(
                    rhs_sb[:, piece * w : (piece + 1) * w],
                    rhs[:, piece * w : (piece + 1) * w],
                ).then_inc(rp[piece], 16)
            for t in range(N_TILES):
                sync.wait_ge(out_sem, t + 1)
                sync.dma_start(
                    out_idx[t * 128 : (t + 1) * 128, :], i8[t % 2][:]
                ).then_inc(dma_out, 16)

        @block.tensor
        def _(tensor):
            tensor.wait_ge(dma_in, 16)
            for k in range(N_CHUNKS):
                t = k // (NS // CHUNK)
                c = k % (NS // CHUNK)
                if k < 2 * (NS // CHUNK):
                    tensor.wait_ge(rp[c // 2], 16)
                if k >= 2:
                    # psum[k%2] free when chunk k-2's readers are done:
                    # Scalar's copy, plus DVE's mixed fold for A-chunks
                    tensor.wait_ge(act_sem, k - 1)
                    if (k - 2) % 2 == 0 and _is_a((k - 2) // 2):
                        tensor.wait_ge(dve_a, _CUM_A[k - 2])
                lt = lhs_sb[:, t * 128 : (t + 1) * 128]
                pt = psum[k % 2]
                for j in range(CHUNK // 256):
                    ins = nc.tensor.matmul(
                        pt[:, j * 256 : (j + 1) * 256],
                        lt,
                        rhs_sb[:, c * CHUNK + j * 256 : c * CHUNK + (j + 1) * 256],
                        start=True,
                        stop=True,
                    )
                    if j == CHUNK // 256 - 1:
                        ins.then_inc(pe_sem, 1)

        @block.scalar
        def _(scalar):
            for k in range(N_CHUNKS):
                p = k // 2
                j = k % 2
                scalar.wait_ge(pe_sem, k + 1)
                if p >= 4 and j == 0:
                    scalar.wait_ge(dve_ing, p - 3)  # cbuf[p%4] consumer done
                if j == 0 and _is_a(p):
                    # A-chunk: copy only the second half; DVE mixed-folds
                    # the psum first half against it
                    ins = nc.scalar.activation(
                        cbuf[p % 4][:, 1024:2048],
                        psum[k % 2][:, 1024:2048],
                        mybir.ActivationFunctionType.Copy,
                    )
                else:
                    ins = nc.scalar.activation(
                        cbuf[p % 4][:, j * CHUNK : (j + 1) * CHUNK],
                        psum[k % 2][:],
                        mybir.ActivationFunctionType.Copy,
                    )
                ins.then_inc(act_sem, 1)

        @block.vector
        def _(vector):
            def fold_a(p):
                """A-pair early op: psum[0:1024] vs copied [1024:2048]."""
                s = p // 2
                vector.wait_ge(act_sem, 2 * p + 1)
                cb = cbuf[p % 4]
                out = f1[s % 2][:, (p % 2) * 2048 : (p % 2) * 2048 + 1024]
                nc.vector.tensor_max(
                    out, psum[0][:, 0:1024], cb[:, 1024:2048]
                ).then_inc(dve_a, 1)

            def fold1(p):
                """ingest pair p (2 chunks) -> f1[super%2] pair-slot (2048)."""
                s = p // 2
                vector.wait_ge(act_sem, 2 * p + 2)
                cb = cbuf[p % 4]
                base = (p % 2) * 2048
                if _is_a(p):
                    # chunk0 already folded by fold_a; fold chunk1 only
                    out = f1[s % 2][:, base + 1024 : base + 2048]
                    ins = nc.vector.tensor_max(
                        out, cb[:, 2048:3072], cb[:, 3072:4096]
                    )
                else:
                    out = pair3(f1[s % 2][:, base : base + 2048], 1024)
                    in0 = pair3(cb[:, 0:PAIR], 1024)
                    in1 = cb[:, 0:PAIR].rearrange("p (c h) -> p c h", c=2)[
                        :, :, 1024:2048
                    ]
                    ins = nc.vector.tensor_max(out, in0, in1)
                ins.then_inc(dve_ing, 1)

            def downstream(s):
                """fold super s's 2x2048 fp16 -> 2x128 slot maxima."""
                a = f1[s % 2][:, 0:4096].rearrange("p (j h) -> p j h", j=2)
                o = f2[s % 2][:, 0:2048].rearrange("p (j h) -> p j h", j=2)
                nc.vector.tensor_max(o, a[:, :, 0:1024], a[:, :, 1024:2048])
                a = f2[s % 2][:, 0:2048].rearrange("p (j h) -> p j h", j=2)
                o = f3[s % 2][:, 0:1024].rearrange("p (j h) -> p j h", j=2)
                nc.vector.tensor_max(o, a[:, :, 0:512], a[:, :, 512:1024])
                a = f3[s % 2][:, 0:1024].rearrange("p (j h) -> p j h", j=2)
                o = f4[s % 2][:, 0:512].rearrange("p (j h) -> p j h", j=2)
                nc.vector.tensor_max(o, a[:, :, 0:256], a[:, :, 256:512])
                a = f4[s % 2][:, 0:512].rearrange("p (j h) -> p j h", j=2)
                o = f5[s % 2][:, 0:256].rearrange("p (j h) -> p j h", j=2)
                nc.vector.tensor_max(o, a[:, :, 0:128], a[:, :, 128:256])

            def maxes(p):
                s = p // 2
                base = (p % 2) * 128
                nc.vector.max(v8a[p % 2][:], f5[s % 2][:, base : base + 64])
                nc.vector.max(v8b[p % 2][:], f5[s % 2][:, base + 64 : base + 128])

            def mindex(p):
                s = p // 2
                t = p // PAIRS_PER_TILE
                c = p % PAIRS_PER_TILE
                base = (p % 2) * 128
                ib = i8[t % 2]
                nc.vector.max_index(
                    ib[:, c * 16 : c * 16 + 8],
                    v8a[p % 2][:],
                    f5[s % 2][:, base : base + 64],
                )
                ins = nc.vector.max_index(
                    ib[:, c * 16 + 8 : c * 16 + 16],
                    v8b[p % 2][:],
                    f5[s % 2][:, base + 64 : base + 128],
                )
                if c == PAIRS_PER_TILE - 1:
                    ins.then_inc(out_sem, 1)

            for s in range(N_SUPERS):
                if _is_a(2 * s):
                    fold_a(2 * s)
                fold1(2 * s)
                if _is_a(2 * s + 1):
                    fold_a(2 * s + 1)
                fold1(2 * s + 1)
                downstream(s)
                if s >= 1:
                    for pp in (2 * s - 2, 2 * s - 1):
                        tp = pp // PAIRS_PER_TILE
                        if pp % PAIRS_PER_TILE == 0 and tp >= 2:
                            vector.wait_ge(dma_out, 16 * (tp - 1))
                        mindex(pp)  # gap after maxes via this super's folds
                maxes(2 * s)
                maxes(2 * s + 1)
            nc.vector.tensor_copy(junk[:], f1[(N_SUPERS - 1) % 2][:, 0:8])  # gap
            mindex(N_PAIRS - 2)
            mindex(N_PAIRS - 1)

    return nc


_NC_CACHE = None


def _get_nc():
    global _NC_CACHE
    if _NC_CACHE is None:
        _NC_CACHE = _build_program()
    return _NC_CACHE


def _exact_d2_rows(q, s_all, cand):
    """Reference-matching fp32 d2 for candidate columns.

    q: (n,3) f32 queries; s_all: (NS,3) f32; cand: (n,m) int
    Returns (n,m) f32 d2 computed as (q_sq + s_sq) - 2*cross, all float32
    like the jax reference.
    """
    q_sq = (q[:, 0] * q[:, 0] + q[:, 1] * q[:, 1]) + q[:, 2] * q[:, 2]
    sc = s_all[cand]  # (n, m, 3)
    s_sq = (sc[..., 0] * sc[..., 0] + sc[..., 1] * sc[..., 1]) + sc[..., 2] * sc[..., 2]
    cross = (q[:, None, 0] * sc[..., 0] + q[:, None, 1] * sc[..., 1]) + (
        q[:, None, 2] * sc[..., 2]
    )
    return (q_sq[:, None] + s_sq) - np.float32(2.0) * cross


def _hilo(x):
    hi = x.astype(np.float16)
    lo = (x - hi.astype(np.float32)).astype(np.float16)
    return hi, lo


def kernel(xyz, xyz_query, n_neighbors):
    global LAST_RESULTS
    xyz = np.asarray(xyz, dtype=np.float32)
    xyz_query = np.asarray(xyz_query, dtype=np.float32)
    k = int(n_neighbors)
    assert k <= 16, f"k={k} too large for candidate margin"

    # --- per-core device inputs (hi/lo residual split, K=11) ---
    # score = (qh+ql) . (sh+sl) - |s_eff|^2/2 with ql*sl dropped (~1e-7):
    # rows: qh*sh (3), ql*sh (3), qh*sl (3), 1*s2h, 1*s2l
    in_maps = []
    for core in range(N_CORES):
        b = core // (N_CORES // B)
        q0 = (core % (N_CORES // B)) * QPC
        q = xyz_query[b, q0 : q0 + QPC]  # (2048, 3)
        s = xyz[b]  # (16384, 3)
        qh, ql = _hilo(q)
        sh, sl = _hilo(s)
        s_eff = sh.astype(np.float64) + sl.astype(np.float64)
        s2 = -0.5 * (s_eff * s_eff).sum(-1)
        s2h = s2.astype(np.float16)
        s2l = (s2 - s2h.astype(np.float64)).astype(np.float16)
        lhsT = np.empty((KDIM, QPC), np.float16)
        rhsm = np.empty((KDIM, NS), np.float16)
        for d in range(3):
            lhsT[d] = qh[:, d]
            rhsm[d] = sh[:, d]
            lhsT[3 + d] = ql[:, d]
            rhsm[3 + d] = sh[:, d]
            lhsT[6 + d] = qh[:, d]
            rhsm[6 + d] = sl[:, d]
        lhsT[9] = 1.0
        rhsm[9] = s2h
        lhsT[10] = 1.0
        rhsm[10] = s2l
        in_maps.append({"lhsT": lhsT, "rhs": rhsm})

    nc = _get_nc()
    res = run_bass_kernel_spmd(nc, in_maps, list(range(N_CORES)))
    LAST_RESULTS = res

    neighbors = np.empty((B, NQ, k), np.int32)
    distances = np.empty((B, NQ, k), np.float32)
    rows_fallback = 0
    stats = [0, 0, 0, 0]  # flag_a, flag_b, flag_c, any-dup counts

    n_win = NS // PAIR  # 4 candidate windows per row
    n_half = 2 * n_win  # 8 half-windows
    j = np.arange(NCAND)
    colbase = (j // 16) * PAIR + ((j % 16) // 8) * 64  # (64,)
    offs = NSLOT * np.arange(FOLD)  # (32,) offsets within a slot's group
    DELTA = np.float32(0.004)

    for core in range(N_CORES):
        b = core // (N_CORES // B)
        q0 = (core % (N_CORES // B)) * QPC
        q = xyz_query[b, q0 : q0 + QPC]
        s = xyz[b]
        r = res.results[core]
        slots = r["out_idx"].astype(np.int64)  # (2048, 64) slot in [0,64)

        # expand each slot to its 32-column fold group
        cand = (
            colbase[None, :, None] + slots[:, :, None] + offs[None, None, :]
        )  # (2048, 64, 32)
        cand2 = cand.reshape(QPC, NCAND * FOLD)
        d2 = _exact_d2_rows(q, s, cand2)  # (2048, 2048) f32

        # top-64 by d2 first (cheap), then stable (d2, idx) order among them
        part = np.argpartition(d2, 63, axis=1)[:, :64]
        d2p = np.take_along_axis(d2, part, 1)
        cp = np.take_along_axis(cand2, part, 1)
        order = np.lexsort((cp, d2p))
        cand_s = np.take_along_axis(cp, order, 1)
        d2_s = np.take_along_axis(d2p, order, 1)
        topk_idx = cand_s[:, :k]
        topk_d2 = d2_s[:, :k]

        # --- conservative fallback detection ---
        thresh = topk_d2[:, k - 1] + DELTA  # (2048,)
        # (a) a half-window contributed >=8 of the found top-k
        half_of = (topk_idx // PAIR) * 2 + (topk_idx % NSLOT) // 64
        counts = (half_of[:, :, None] == np.arange(n_half)[None, None]).sum(1)
        flag_a = counts.max(1) >= 8
        # (b) slot boundary within noise of the found k-th d2
        gmin = d2.reshape(QPC, NCAND, FOLD).min(2)  # (2048, 64) slot-group best
        boundary = gmin.reshape(QPC, n_half, 8).max(2)  # (2048, 8)
        flag_b = (boundary.min(1) - topk_d2[:, k - 1]) < DELTA
        # (c) duplicate slots within a half-window (max_index tie artifact):
        #     only matters if the dup slot's group-best is near the k-th d2
        sh_ = slots.reshape(QPC, n_half, 8)
        dup = (sh_[:, :, :, None] == sh_[:, :, None, :]).sum(3) > 1  # (q,h,8)
        gmin_h = gmin.reshape(QPC, n_half, 8)
        dup_gmin = np.where(dup, gmin_h, np.float32(np.inf)).min((1, 2))
        flag_c = dup_gmin < thresh
        flag = flag_a | flag_b | flag_c
        stats[0] += int(flag_a.sum())
        stats[1] += int(flag_b.sum())
        stats[2] += int(flag_c.sum())
        stats[3] += int(dup.any((1, 2)).sum())

        nb = topk_idx.astype(np.int32)
        dd = topk_d2

        if flag.any():
            rows = np.nonzero(flag)[0]
            rows_fallback += len(rows)
            full = _exact_d2_rows(
                q[rows], s, np.broadcast_to(np.arange(NS), (len(rows), NS))
            )
            forder = np.lexsort((np.broadcast_to(np.arange(NS), full.shape), full))
            nb[rows] = forder[:, :k].astype(np.int32)
            dd = dd.copy()
            dd[rows] = np.take_along_axis(full, forder[:, :k], 1)

        neighbors[b, q0 : q0 + QPC] = nb
        distances[b, q0 : q0 + QPC] = np.sqrt(np.maximum(dd, np.float32(0.0)))

    kernel.rows_fallback = rows_fallback
    kernel.flag_stats = tuple(stats)
    return neighbors, distances


# revision 30
# speedup vs baseline: 1.7586x; 1.0213x over previous
"""Brute-force KNN (B=2, Ns=16384, Nq=8192, d=3, k<=16) on 8 trn2 NeuronCores.

Strategy (data-parallel over queries):
  - 16384 total queries sharded 2048/core (cores 0-3: batch 0, cores 4-7: batch 1).
  - PE computes score[q,s] = q . s - ||s||^2/2 (rank-equivalent to -d2/2) via
    a K=11 fp16 matmul built from hi/lo residual splits of the coordinates
    and the ||s||^2 term, so the fp32 PSUM scores are exact to ~1e-6 for the
    fp32 inputs (matmul time on this hw is K-independent). 2048 support
    columns per chunk (psum [128,2048]f32 x2 = all 8 banks), 256-col matmul
    instructions (measured 2x the per-column rate of 512-col ones).
  - Top-k on trn2 is bottlenecked by reading PSUM: Max/MaxIndex run at
    1 elem/lane/cycle with no fast modes, and TensorTensor cannot read two
    PSUM operands. So the Scalar engine (Activation Copy, ~0.83ns/elem)
    drains every psum chunk to SBUF fp16 (the pipeline's rate limiter at
    ~252us) while the DVE max-folds 32:1 down to 128 fp16 slot maxima per
    4096-col window (2-byte 2x mode). Fold ops are merged across chunk pairs
    and pair "supers" via strided APs to amortize per-instruction overhead.
    Max/MaxIndex then pick the top-8 slots of each 64-slot half-window (64
    candidates/query). PE (~218us) and DVE (~230us) hide behind Scalar.
  - Each slot names a group of 32 support columns (col = win*4096 + slot +
    128*j). The host exact-reranks all 64*32=2048 columns in fp32 - fold
    losers are recovered because a fold winner always beats its group, so
    any true neighbor's group winner is itself a top candidate.
  - Conservative host fallbacks (full-row exact rerank) for: a half-window
    holding >=8 of the found top-k, slot-boundary within fold-fp16 noise
    (delta=0.004) of the found k-th distance, duplicated slots near the
    boundary.
"""

import numpy as np

import concourse.bass as bass
from concourse import mybir
from concourse.bass_utils import run_bass_kernel_spmd

B = 2
NS = 16384
NQ = 8192
N_CORES = 8
QPC = (B * NQ) // N_CORES  # queries per core = 2048
N_TILES = QPC // 128  # 16
KDIM = 11  # hi/lo residual rows
CHUNK = 2048  # psum chunk ([128,2048] fp32 = 4 banks, x2 buffers = all PSUM)
PAIR = 2 * CHUNK  # candidate window: two psum chunks fold into one 4096-window
PAIRS_PER_TILE = NS // PAIR  # 4
N_CHUNKS = N_TILES * (NS // CHUNK)  # 128
N_PAIRS = N_CHUNKS // 2  # 64
N_SUPERS = N_PAIRS // 2  # 32 (two pairs share merged downstream folds)
FOLD = 32  # 4096-window -> 128 slots
NSLOT = PAIR // FOLD  # 128
NCAND = PAIRS_PER_TILE * 16  # 64 slots per query (top-8 of each 64-slot half)

# A-pairs: for the even chunk, Scalar copies only its second half and the DVE
# max-folds the psum first half against that copy (mixed fold, scheduled
# early so the psum recycle latency stays short). Balances Scalar vs DVE.
A_COUNT = 0


def _is_a(p: int) -> bool:
    return (p * A_COUNT) // N_PAIRS != ((p + 1) * A_COUNT) // N_PAIRS


_CUM_A = []
_c = 0
for _k in range(N_CHUNKS):
    _c += 1 if (_k % 2 == 0 and _is_a(_k // 2)) else 0
    _CUM_A.append(_c)

LAST_RESULTS = None  # stashed BassKernelResults for test harness introspection


def _build_program():
    nc = bass.Bass()
    lhsT = nc.declare_dram_parameter(
        "lhsT", [KDIM, QPC], mybir.dt.float16, isOutput=False
    )
    rhs = nc.declare_dram_parameter("rhs", [KDIM, NS], mybir.dt.float16, isOutput=False)
    out_idx = nc.declare_dram_parameter(
        "out_idx", [QPC, NCAND], mybir.dt.uint16, isOutput=True
    )

    from contextlib import ExitStack

    with ExitStack() as stack:
        _n = [0]

        def sb(shape, dt):
            _n[0] += 1
            return stack.enter_context(nc.sbuf_tensor(f"sb{_n[0]}", shape, dt))

        lhs_sb = sb([KDIM, QPC], mybir.dt.float16)
        rhs_sb = sb([KDIM, NS], mybir.dt.float16)
        psum = [
            stack.enter_context(
                nc.psum_tensor(f"ps{i}", [128, CHUNK], mybir.dt.float32)
            )
            for i in range(2)
        ]
        cbuf = [sb([128, PAIR], mybir.dt.float16) for _ in range(4)]  # per-pair ring
        f1 = [sb([128, 4096], mybir.dt.float16) for _ in range(2)]  # per-super ring
        f2 = [sb([128, 2048], mybir.dt.float16) for _ in range(2)]
        f3 = [sb([128, 1024], mybir.dt.float16) for _ in range(2)]
        f4 = [sb([128, 512], mybir.dt.float16) for _ in range(2)]
        f5 = [sb([128, 256], mybir.dt.float16) for _ in range(2)]
        v8a = [sb([128, 8], mybir.dt.float16) for _ in range(2)]
        v8b = [sb([128, 8], mybir.dt.float16) for _ in range(2)]
        i8 = [sb([128, NCAND], mybir.dt.uint16) for _ in range(2)]
        junk = sb([128, 8], mybir.dt.float16)
        sem = lambda name: stack.enter_context(nc.semaphore(name))
        dma_in = sem("dma_in")
        rp = [sem(f"rp{i}") for i in range(8)]
        pe_sem = sem("pe_sem")
        act_sem = sem("act_sem")
        dve_ing = sem("dve_ing")
        dve_a = sem("dve_a")
        out_sem = sem("out_sem")
        dma_out = sem("dma_out")
        block = stack.enter_context(nc.Block())

        def pair3(ap, h):
            """[128, 2*n] flat -> [128, 2, h] strided view (n = ap cols / 2)."""
            return ap.rearrange("p (c h) -> p c h", c=2)[:, :, 0:h]

        @block.sync
        def _(sync):
            sync.dma_start(lhs_sb[:], lhsT[:]).then_inc(dma_in, 16)
            for piece in range(8):
                w = NS // 8
                sync.dma_start(
                    rhs_sb[:, piece * w : (piece + 1) * w],
                    rhs[:, piece * w : (piece + 1) * w],
                ).then_inc(rp[piece], 16)
            for t in range(N_TILES):
                sync.wait_ge(out_sem, t + 1)
                sync.dma_start(
                    out_idx[t * 128 : (t + 1) * 128, :], i8[t % 2][:]
                ).then_inc(dma_out, 16)

        @block.tensor
        def _(tensor):
            tensor.wait_ge(dma_in, 16)
            for k in range(N_CHUNKS):
                t = k // (NS // CHUNK)
                c = k % (NS // CHUNK)
                if k < 2 * (NS // CHUNK):
                    tensor.wait_ge(rp[c], 16)
                if k >= 2:
                    # psum[k%2] free when chunk k-2's readers are done:
                    # Scalar's copy, plus DVE's mixed fold for A-chunks
                    tensor.wait_ge(act_sem, k - 1)
                    if (k - 2) % 2 == 0 and _is_a((k - 2) // 2):
                        tensor.wait_ge(dve_a, _CUM_A[k - 2])
                lt = lhs_sb[:, t * 128 : (t + 1) * 128]
                pt = psum[k % 2]
                for j in range(CHUNK // 256):
                    ins = nc.tensor.matmul(
                        pt[:, j * 256 : (j + 1) * 256],
                        lt,
                        rhs_sb[:, c * CHUNK + j * 256 : c * CHUNK + (j + 1) * 256],
                        start=True,
                        stop=True,
                    )
                    if j == CHUNK // 256 - 1:
                        ins.then_inc(pe_sem, 1)

        @block.scalar
        def _(scalar):
            for k in range(N_CHUNKS):
                p = k // 2
                j = k % 2
                scalar.wait_ge(pe_sem, k + 1)
                if p >= 4 and j == 0:
                    scalar.wait_ge(dve_ing, p - 3)  # cbuf[p%4] consumer done
                if j == 0 and _is_a(p):
                    # A-chunk: copy only the second half; DVE mixed-folds
                    # the psum first half against it
                    ins = nc.scalar.activation(
                        cbuf[p % 4][:, 1024:2048],
                        psum[k % 2][:, 1024:2048],
                        mybir.ActivationFunctionType.Copy,
                    )
                else:
                    ins = nc.scalar.activation(
                        cbuf[p % 4][:, j * CHUNK : (j + 1) * CHUNK],
                        psum[k % 2][:],
                        mybir.ActivationFunctionType.Copy,
                    )
                ins.then_inc(act_sem, 1)

        @block.vector
        def _(vector):
            def fold_a(p):
                """A-pair early op: psum[0:1024] vs copied [1024:2048]."""
                s = p // 2
                vector.wait_ge(act_sem, 2 * p + 1)
                cb = cbuf[p % 4]
                out = f1[s % 2][:, (p % 2) * 2048 : (p % 2) * 2048 + 1024]
                nc.vector.tensor_max(
                    out, psum[0][:, 0:1024], cb[:, 1024:2048]
                ).then_inc(dve_a, 1)

            def fold1(p):
                """ingest pair p (2 chunks) -> f1[super%2] pair-slot (2048)."""
                s = p // 2
                vector.wait_ge(act_sem, 2 * p + 2)
                cb = cbuf[p % 4]
                base = (p % 2) * 2048
                if _is_a(p):
                    # chunk0 already folded by fold_a; fold chunk1 only
                    out = f1[s % 2][:, base + 1024 : base + 2048]
                    ins = nc.vector.tensor_max(
                        out, cb[:, 2048:3072], cb[:, 3072:4096]
                    )
                else:
                    out = pair3(f1[s % 2][:, base : base + 2048], 1024)
                    in0 = pair3(cb[:, 0:PAIR], 1024)
                    in1 = cb[:, 0:PAIR].rearrange("p (c h) -> p c h", c=2)[
                        :, :, 1024:2048
                    ]
                    ins = nc.vector.tensor_max(out, in0, in1)
                ins.then_inc(dve_ing, 1)

            def downstream(s):
                """fold super s's 2x2048 fp16 -> 2x128 slot maxima."""
                a = f1[s % 2][:, 0:4096].rearrange("p (j h) -> p j h", j=2)
                o = f2[s % 2][:, 0:2048].rearrange("p (j h) -> p j h", j=2)
                nc.vector.tensor_max(o, a[:, :, 0:1024], a[:, :, 1024:2048])
                a = f2[s % 2][:, 0:2048].rearrange("p (j h) -> p j h", j=2)
                o = f3[s % 2][:, 0:1024].rearrange("p (j h) -> p j h", j=2)
                nc.vector.tensor_max(o, a[:, :, 0:512], a[:, :, 512:1024])
                a = f3[s % 2][:, 0:1024].rearrange("p (j h) -> p j h", j=2)
                o = f4[s % 2][:, 0:512].rearrange("p (j h) -> p j h", j=2)
                nc.vector.tensor_max(o, a[:, :, 0:256], a[:, :, 256:512])
                a = f4[s % 2][:, 0:512].rearrange("p (j h) -> p j h", j=2)
                o = f5[s % 2][:, 0:256].rearrange("p (j h) -> p j h", j=2)
                nc.vector.tensor_max(o, a[:, :, 0:128], a[:, :, 128:256])

            def maxes(p):
                s = p // 2
                base = (p % 2) * 128
                nc.vector.max(v8a[p % 2][:], f5[s % 2][:, base : base + 64])
                nc.vector.max(v8b[p % 2][:], f5[s % 2][:, base + 64 : base + 128])

            def mindex(p):
                s = p // 2
                t = p // PAIRS_PER_TILE
                c = p % PAIRS_PER_TILE
                base = (p % 2) * 128
                ib = i8[t % 2]
                nc.vector.max_index(
                    ib[:, c * 16 : c * 16 + 8],
                    v8a[p % 2][:],
                    f5[s % 2][:, base : base + 64],
                )
                ins = nc.vector.max_index(
                    ib[:, c * 16 + 8 : c * 16 + 16],
                    v8b[p % 2][:],
                    f5[s % 2][:, base + 64 : base + 128],
                )
                if c == PAIRS_PER_TILE - 1:
                    ins.then_inc(out_sem, 1)

            for s in range(N_SUPERS):
                if _is_a(2 * s):
                    fold_a(2 * s)
                fold1(2 * s)
                if _is_a(2 * s + 1):
                    fold_a(2 * s + 1)
                fold1(2 * s + 1)
                downstream(s)
                if s >= 1:
                    for pp in (2 * s - 2, 2 * s - 1):
                        tp = pp // PAIRS_PER_TILE
                        if pp % PAIRS_PER_TILE == 0 and tp >= 2:
                            vector.wait_ge(dma_out, 16 * (tp - 1))
                        mindex(pp)  # gap after maxes via this super's folds
                maxes(2 * s)
                maxes(2 * s + 1)
            nc.vector.tensor_copy(junk[:], f1[(N_SUPERS - 1) % 2][:, 0:8])  # gap
            mindex(N_PAIRS - 2)
            mindex(N_PAIRS - 1)

    return nc


_NC_CACHE = None


def _get_nc():
    global _NC_CACHE
    if _NC_CACHE is None:
        _NC_CACHE = _build_program()
    return _NC_CACHE


def _exact_d2_rows(q, s_all, cand):
    """Reference-matching fp32 d2 for candidate columns.

    q: (n,3) f32 queries; s_all: (NS,3) f32; cand: (n,m) int
    Returns (n,m) f32 d2 computed as (q_sq + s_sq) - 2*cross, all float32
    like the jax reference.
    """
    q_sq = (q[:, 0] * q[:, 0] + q[:, 1] * q[:, 1]) + q[:, 2] * q[:, 2]
    sc = s_all[cand]  # (n, m, 3)
    s_sq = (sc[..., 0] * sc[..., 0] + sc[..., 1] * sc[..., 1]) + sc[..., 2] * sc[..., 2]
    cross = (q[:, None, 0] * sc[..., 0] + q[:, None, 1] * sc[..., 1]) + (
        q[:, None, 2] * sc[..., 2]
    )
    return (q_sq[:, None] + s_sq) - np.float32(2.0) * cross


def _hilo(x):
    hi = x.astype(np.float16)
    lo = (x - hi.astype(np.float32)).astype(np.float16)
    return hi, lo


def kernel(xyz, xyz_query, n_neighbors):
    global LAST_RESULTS
    xyz = np.asarray(xyz, dtype=np.float32)
    xyz_query = np.asarray(xyz_query, dtype=np.float32)
    k = int(n_neighbors)
    assert k <= 16, f"k={k} too large for candidate margin"

    # --- per-core device inputs (hi/lo residual split, K=11) ---
    # score = (qh+ql) . (sh+sl) - |s_eff|^2/2 with ql*sl dropped (~1e-7):
    # rows: qh*sh (3), ql*sh (3), qh*sl (3), 1*s2h, 1*s2l
    in_maps = []
    for core in range(N_CORES):
        b = core // (N_CORES // B)
        q0 = (core % (N_CORES // B)) * QPC
        q = xyz_query[b, q0 : q0 + QPC]  # (2048, 3)
        s = xyz[b]  # (16384, 3)
        qh, ql = _hilo(q)
        sh, sl = _hilo(s)
        s_eff = sh.astype(np.float64) + sl.astype(np.float64)
        s2 = -0.5 * (s_eff * s_eff).sum(-1)
        s2h = s2.astype(np.float16)
        s2l = (s2 - s2h.astype(np.float64)).astype(np.float16)
        lhsT = np.empty((KDIM, QPC), np.float16)
        rhsm = np.empty((KDIM, NS), np.float16)
        for d in range(3):
            lhsT[d] = qh[:, d]
            rhsm[d] = sh[:, d]
            lhsT[3 + d] = ql[:, d]
            rhsm[3 + d] = sh[:, d]
            lhsT[6 + d] = qh[:, d]
            rhsm[6 + d] = sl[:, d]
        lhsT[9] = 1.0
        rhsm[9] = s2h
        lhsT[10] = 1.0
        rhsm[10] = s2l
        in_maps.append({"lhsT": lhsT, "rhs": rhsm})

    nc = _get_nc()
    res = run_bass_kernel_spmd(nc, in_maps, list(range(N_CORES)))
    LAST_RESULTS = res

    neighbors = np.empty((B, NQ, k), np.int32)
    distances = np.empty((B, NQ, k), np.float32)
    rows_fallback = 0
    stats = [0, 0, 0, 0]  # flag_a, flag_b, flag_c, any-dup counts

    n_win = NS // PAIR  # 4 candidate windows per row
    n_half = 2 * n_win  # 8 half-windows
    j = np.arange(NCAND)
    colbase = (j // 16) * PAIR + ((j % 16) // 8) * 64  # (64,)
    offs = NSLOT * np.arange(FOLD)  # (32,) offsets within a slot's group
    DELTA = np.float32(0.004)

    for core in range(N_CORES):
        b = core // (N_CORES // B)
        q0 = (core % (N_CORES // B)) * QPC
        q = xyz_query[b, q0 : q0 + QPC]
        s = xyz[b]
        r = res.results[core]
        slots = r["out_idx"].astype(np.int64)  # (2048, 64) slot in [0,64)

        # expand each slot to its 32-column fold group
        cand = (
            colbase[None, :, None] + slots[:, :, None] + offs[None, None, :]
        )  # (2048, 64, 32)
        cand2 = cand.reshape(QPC, NCAND * FOLD)
        d2 = _exact_d2_rows(q, s, cand2)  # (2048, 2048) f32

        # top-64 by d2 first (cheap), then stable (d2, idx) order among them
        part = np.argpartition(d2, 63, axis=1)[:, :64]
        d2p = np.take_along_axis(d2, part, 1)
        cp = np.take_along_axis(cand2, part, 1)
        order = np.lexsort((cp, d2p))
        cand_s = np.take_along_axis(cp, order, 1)
        d2_s = np.take_along_axis(d2p, order, 1)
        topk_idx = cand_s[:, :k]
        topk_d2 = d2_s[:, :k]

        # --- conservative fallback detection ---
        thresh = topk_d2[:, k - 1] + DELTA  # (2048,)
        # (a) a half-window contributed >=8 of the found top-k
        half_of = (topk_idx // PAIR) * 2 + (topk_idx % NSLOT) // 64
        counts = (half_of[:, :, None] == np.arange(n_half)[None, None]).sum(1)
        flag_a = counts.max(1) >= 8
        # (b) slot boundary within noise of the found k-th d2
        gmin = d2.reshape(QPC, NCAND, FOLD).min(2)  # (2048, 64) slot-group best
        boundary = gmin.reshape(QPC, n_half, 8).max(2)  # (2048, 8)
        flag_b = (boundary.min(1) - topk_d2[:, k - 1]) < DELTA
        # (c) duplicate slots within a half-window (max_index tie artifact):
        #     only matters if the dup slot's group-best is near the k-th d2
        sh_ = slots.reshape(QPC, n_half, 8)
        dup = (sh_[:, :, :, None] == sh_[:, :, None, :]).sum(3) > 1  # (q,h,8)
        gmin_h = gmin.reshape(QPC, n_half, 8)
        dup_gmin = np.where(dup, gmin_h, np.float32(np.inf)).min((1, 2))
        flag_c = dup_gmin < thresh
        flag = flag_a | flag_b | flag_c
        stats[0] += int(flag_a.sum())
        stats[1] += int(flag_b.sum())
        stats[2] += int(flag_c.sum())
        stats[3] += int(dup.any((1, 2)).sum())

        nb = topk_idx.astype(np.int32)
        dd = topk_d2

        if flag.any():
            rows = np.nonzero(flag)[0]
            rows_fallback += len(rows)
            full = _exact_d2_rows(
                q[rows], s, np.broadcast_to(np.arange(NS), (len(rows), NS))
            )
            forder = np.lexsort((np.broadcast_to(np.arange(NS), full.shape), full))
            nb[rows] = forder[:, :k].astype(np.int32)
            dd = dd.copy()
            dd[rows] = np.take_along_axis(full, forder[:, :k], 1)

        neighbors[b, q0 : q0 + QPC] = nb
        distances[b, q0 : q0 + QPC] = np.sqrt(np.maximum(dd, np.float32(0.0)))

    kernel.rows_fallback = rows_fallback
    kernel.flag_stats = tuple(stats)
    return neighbors, distances
